# revision 3
# baseline (speedup 1.0000x reference)
"""Trainium2 Bass kernel for nn_DecGreenNet_product_CP2 (8-core SPMD).

Math (algebraically identical to the reference; numpy-verified ~7e-4 rel
with the mixed fp32/fp16/bf16 precision used here):
    y_b  = exp(-eq * ||q_b||^2)                          per quad point
    t_b  = sum_p y_b[p] * tanh(q_b[p] @ Wq_ba + bq_ba)   [1024]
    s_b  = sum_p y_b[p]
    S_b  = t_b @ Wq_bb + s_b * bq_bb                     [64,64] (flat 4096)
    v    = vec(S_0 @ S_1^T)                              [4096]
    w    = Wx2 @ v, c = bx2 . v
    out  = tanh(X @ Wx1 + bx1) @ w + c                   [16384]
This avoids materializing the reference's 16384x4096 `lhs` (137 GFLOP ->
~0.2 GFLOP).

Sharding over 8 cores:
  - main branch: data-parallel over the 16384 rows (2048/core)
  - quad branch: sharded over the hidden dim k (128/core, all 4096 points)
    -> per-core t-chunks are exact; partial S^T needs one AllReduce (32KB)
  - w: Wx2 row-chunk per core -> exact w chunk -> one AllGather (512B/core)

Precision: tanh stores fp16 (bounded in [-1,1]); w scaled by 1/4096 into
fp16 (unscaled in the final fused multiply-add); quad tanh + y in bf16
(averaged over 4096 points); everything else fp32.
"""

import numpy as np
import ml_dtypes

import concourse.bacc as bacc
import concourse.bass as bass
import concourse.mybir as mybir
import concourse.tile as tile
from concourse import bass_utils

N_CORES = 8
N, D_IN = 16384, 3
HID = 1024
R = 64
RR = 4096
NQ = 4096
NPC = N // N_CORES
KC = HID // N_CORES
P = 128
F32 = mybir.dt.float32
BF16 = mybir.dt.bfloat16
F16 = mybir.dt.float16

AF = mybir.ActivationFunctionType
ALU = mybir.AluOpType


def build_module():
    nc = bacc.Bacc(
        "TRN2",
        target_bir_lowering=False,
        debug=False,
        enable_asserts=False,
        num_devices=N_CORES,
    )

    def din(name, shape, dt=F32):
        return nc.dram_tensor(name, shape, dt, kind="ExternalInput").ap()

    xT1 = din("xT1", [4, NPC], F16)
    w1b = din("w1b", [4, HID], F16)
    qT1 = din("qT1", [8, NQ])
    wqab = din("wqab", [8, 2 * KC])
    qq0 = din("qq0", [P, 96])
    qq1 = din("qq1", [P, 96])
    eqb = din("eqb", [P, 1])
    wq0b = din("wq0b", [P, RR])
    wq1b = din("wq1b", [P, RR])
    bq0Tm = din("bq0Tm", [R, R])
    bq1Tm = din("bq1Tm", [R, R])
    wx2T = din("wx2T", [RR, P])
    bx2m = din("bx2m", [P, 32])
    outp = nc.dram_tensor("outp", [P, NPC // P], F32, kind="ExternalOutput").ap()

    with tile.TileContext(nc) as tc:
        with (
            tc.tile_pool(name="sbw", bufs=1) as sbw,
            tc.tile_pool(name="sbact", bufs=1) as sbact,
            tc.tile_pool(name="stage", bufs=2) as stage,
            tc.tile_pool(name="sbs", bufs=1) as sbs,
            tc.tile_pool(name="psbig", bufs=3, space="PSUM") as psbig,
            tc.tile_pool(name="pssm", bufs=2, space="PSUM") as pssm,
            tc.tile_pool(name="dram", bufs=1, space="DRAM") as dram,
        ):
            # ---------- input DMAs ----------
            xT1_sb = sbw.tile([4, NPC], F16)
            nc.sync.dma_start(xT1_sb[:], xT1[:])
            w1b_sb = sbw.tile([4, HID], F16)
            nc.sync.dma_start(w1b_sb[:], w1b[:])
            qT1_sb = sbw.tile([8, NQ], F32)
            nc.sync.dma_start(qT1_sb[:], qT1[:])
            wqab_sb = sbw.tile([8, 2 * KC], F32)
            nc.sync.dma_start(wqab_sb[:], wqab[:])
            qq0_sb = sbw.tile([P, 96], F32)
            nc.sync.dma_start(qq0_sb[:], qq0[:])
            qq1_sb = sbw.tile([P, 96], F32)
            nc.sync.dma_start(qq1_sb[:], qq1[:])
            eqb_sb = sbw.tile([P, 1], F32)
            nc.sync.dma_start(eqb_sb[:], eqb[:])
            bq0Tm_sb = sbw.tile([R, R], F32)
            nc.sync.dma_start(bq0Tm_sb[:], bq0Tm[:])
            bq1Tm_sb = sbw.tile([R, R], F32)
            nc.sync.dma_start(bq1Tm_sb[:], bq1Tm[:])
            bx2m_sb = sbw.tile([P, 32], F32)
            nc.sync.dma_start(bx2m_sb[:], bx2m[:])

            wq0b_sb = sbw.tile([P, RR], F32)
            wq1b_sb = sbw.tile([P, RR], F32)
            for j in range(8):
                cs = slice(512 * j, 512 * (j + 1))
                nc.sync.dma_start(wq0b_sb[:, cs], wq0b[:, cs])
            for j in range(8):
                cs = slice(512 * j, 512 * (j + 1))
                nc.sync.dma_start(wq1b_sb[:, cs], wq1b[:, cs])
            wx2T_sb = sbw.tile([P, RR], F32)
            wx2T_r = wx2T.rearrange("(t p) m -> p t m", p=P)
            wx2T_sb_r = wx2T_sb.rearrange("p (t m) -> p t m", m=P)
            for j in range(8):
                nc.sync.dma_start(
                    wx2T_sb_r[:, 4 * j : 4 * (j + 1), :],
                    wx2T_r[:, 4 * j : 4 * (j + 1), :],
                )

            # ones vectors for PE-based partition reductions / broadcasts
            ones_p = sbs.tile([P, 1], F32)
            nc.vector.memset(ones_p[:], 1.0)
            ones_r1 = sbs.tile([1, P], F32)
            nc.vector.memset(ones_r1[:], 1.0)

            # ---------- quadrature weights y_b ----------
            neq = sbs.tile([P, 1], F32)
            nc.vector.tensor_scalar_mul(neq[:], eqb_sb[:], -1.0)

            ys = []
            svs = []
            for bi, qq_sb in enumerate((qq0_sb, qq1_sb)):
                sq = sbs.tile([P, 96], F32, name=f"sq{bi}")
                nc.vector.tensor_tensor(sq[:], qq_sb[:], qq_sb[:], ALU.mult)
                ss = sbs.tile([P, 32], F32, name=f"ss{bi}")
                sq3 = sq.rearrange("p (t d) -> p t d", d=3)
                nc.vector.tensor_tensor(ss[:], sq3[:, :, 0], sq3[:, :, 1], ALU.add)
                nc.vector.tensor_tensor(ss[:], ss[:], sq3[:, :, 2], ALU.add)
                y = sbs.tile([P, 32], F32, name=f"y{bi}")
                nc.scalar.activation(y[:], ss[:], AF.Exp, scale=neq[:])
                yb = sbs.tile([P, 32], BF16, name=f"yb{bi}")
                nc.vector.tensor_copy(yb[:], y[:])
                # s_b = sum(y): partition-sum via PE, then broadcast to 64 parts
                sp_ps = pssm.tile([1, 32], F32, tag="sm", name=f"spps{bi}")
                nc.tensor.matmul(sp_ps[:], ones_p[:], y[:], start=True, stop=True)
                srow = sbs.tile([1, 32], F32, name=f"srow{bi}")
                nc.vector.tensor_copy(srow[:], sp_ps[:])
                ssc = sbs.tile([1, 1], F32, name=f"ssc{bi}")
                nc.vector.tensor_reduce(ssc[:], srow[:], axis=mybir.AxisListType.X,
                                        op=ALU.add)
                sb_ps = pssm.tile([R, 1], F32, tag="sm", name=f"sbps{bi}")
                nc.tensor.matmul(sb_ps[:], ones_r1[:, :R], ssc[:], start=True,
                                 stop=True)
                sv = sbs.tile([R, 1], F32, name=f"sv{bi}")
                nc.vector.tensor_copy(sv[:], sb_ps[:])
                ys.append(yb)
                svs.append(sv)

            # ---------- quad hidden (psum) -> DVE stage -> wide tanh ----------
            # psum round r in 0..7: [128, 1024] = 4 pt-tiles x [128, 256]
            # stage tile g in 0..1: [128, 4096] = rounds 4g..4g+3
            # qt[g] bf16 [128, 4096]; slice for (ptile j, br): cols
            #   1024*(j//4 - 4g) + 256*(j%4) + 128*br
            qt = []
            for g in range(2):
                stg = stage.tile([P, 4096], F32, tag="stg", name=f"qstg{g}")
                for rr in range(4):
                    r = 4 * g + rr
                    q_ps = psbig.tile([P, 1024], F32, tag="big", name=f"qps{r}")
                    for pt in range(4):
                        j = 4 * r + pt
                        nc.tensor.matmul(
                            q_ps[:, 256 * pt : 256 * (pt + 1)],
                            qT1_sb[:, P * j : P * (j + 1)],
                            wqab_sb[:],
                            start=True,
                            stop=True,
                        )
                    nc.vector.tensor_copy(stg[:, 1024 * rr : 1024 * (rr + 1)],
                                          q_ps[:])
                qtg = sbact.tile([P, 4096], BF16, name=f"qt{g}")
                nc.scalar.activation(qtg[:], stg[:], AF.Tanh)
                qt.append(qtg)

            # ---------- t-chunks ----------
            t_sb = []
            for br in range(2):
                t_ps = pssm.tile([P, 1], F32, tag="sm", name=f"tps{br}")
                for j in range(32):
                    off = 1024 * ((j // 4) % 4) + 256 * (j % 4) + KC * br
                    nc.tensor.matmul(
                        t_ps[:],
                        qt[j // 16][:, off : off + KC],
                        ys[br][:, j : j + 1],
                        start=(j == 0),
                        stop=(j == 31),
                    )
                t1 = sbs.tile([P, 1], F32, name=f"t{br}")
                nc.vector.tensor_copy(t1[:], t_ps[:])
                t_sb.append(t1)

            # ---------- main hidden -> stage -> wide tanh ----------
            # round idx = 2m + h (m = k-chunk, h = pts-half)
            # stage tile g holds rounds 4g..4g+3; ht[g] bf16 [128, 4096]
            ht = []
            for g in range(4):
                stg = stage.tile([P, 4096], F32, tag="stg", name=f"hstg{g}")
                for i in range(4):
                    ridx = 4 * g + i
                    m, h = divmod(ridx, 2)
                    hx_ps = psbig.tile([P, 1024], F32, tag="big",
                                       name=f"hxps{ridx}")
                    for s in range(2):
                        nc.tensor.matmul(
                            hx_ps[:, 512 * s : 512 * (s + 1)],
                            w1b_sb[:, P * m : P * (m + 1)],
                            xT1_sb[:, 1024 * h + 512 * s : 1024 * h + 512 * (s + 1)],
                            start=True,
                            stop=True,
                        )
                    nc.vector.tensor_copy(stg[:, 1024 * i : 1024 * (i + 1)],
                                          hx_ps[:])
                htg = sbact.tile([P, 4096], F16, name=f"ht{g}")
                nc.scalar.activation(htg[:], stg[:], AF.Tanh)
                ht.append(htg)

            # ---------- partial S^T ----------
            ar1_in = dram.tile([R, 2 * R], F32)
            ar1_out = dram.tile([R, 2 * R], F32, addr_space="Shared")
            for br, wqb_sb in enumerate((wq0b_sb, wq1b_sb)):
                st = sbs.tile([R, R], F32, name=f"STp{br}")
                for g in range(8):
                    s_ps = pssm.tile([R, 8], F32, tag="sm", name=f"sps{br}_{g}")
                    for jj in range(8):
                        j = 8 * g + jj
                        nc.tensor.matmul(
                            s_ps[:, jj : jj + 1],
                            wqb_sb[:, R * j : R * (j + 1)],
                            t_sb[br][:],
                            start=True,
                            stop=True,
                        )
                    nc.vector.tensor_copy(st[:, 8 * g : 8 * (g + 1)], s_ps[:])
                nc.sync.dma_start(ar1_in[:, R * br : R * (br + 1)], st[:])

            nc.gpsimd.collective_compute(
                "AllReduce",
                ALU.add,
                replica_groups=[list(range(N_CORES))],
                ins=[ar1_in[:]],
                outs=[ar1_out[:]],
            )

            # ---------- S^T + bias; v ----------
            STf = []
            for br, bqTm_sb in enumerate((bq0Tm_sb, bq1Tm_sb)):
                stl = sbs.tile([R, R], F32, name=f"STl{br}")
                nc.sync.dma_start(stl[:], ar1_out[:, R * br : R * (br + 1)])
                tmpb = sbs.tile([R, R], F32, name=f"tmpb{br}")
                nc.vector.tensor_scalar(tmpb[:], bqTm_sb[:], svs[br][:], None,
                                        ALU.mult)
                nc.vector.tensor_tensor(stl[:], stl[:], tmpb[:], ALU.add)
                STf.append(stl)

            v_ps = pssm.tile([P, 32], F32, tag="sm")
            S0T, S1T = STf
            S0T_r = S0T.rearrange("x (t two) -> x t two", two=2)
            nc.tensor.matmul(v_ps[:R, :], S1T[:], S0T_r[:, :, 0], start=True,
                             stop=True)
            nc.tensor.matmul(v_ps[R:, :], S1T[:], S0T_r[:, :, 1], start=True,
                             stop=True)
            v_sb = sbs.tile([P, 32], F32)
            nc.vector.tensor_copy(v_sb[:], v_ps[:])

            # ---------- c = bx2 . v ----------
            dumm = sbs.tile([P, 1], F32)
            cpart = sbs.tile([P, 1], F32)
            nc.vector.tensor_tensor_reduce(
                dumm.broadcast_to([P, 32]),
                v_sb[:],
                bx2m_sb[:],
                scale=1.0,
                scalar=0.0,
                op0=ALU.mult,
                op1=ALU.add,
                accum_out=cpart[:],
            )
            # c = sum_p cpart: partition-sum via PE, broadcast to 128 parts
            c1_ps = pssm.tile([1, 1], F32, tag="sm")
            nc.tensor.matmul(c1_ps[:], cpart[:], ones_p[:], start=True, stop=True)
            c_sc = sbs.tile([1, 1], F32)
            nc.vector.tensor_copy(c_sc[:], c1_ps[:])
            cb_ps = pssm.tile([P, 1], F32, tag="sm")
            nc.tensor.matmul(cb_ps[:], ones_r1[:], c_sc[:], start=True, stop=True)
            c_all = sbs.tile([P, 1], F32)
            nc.vector.tensor_copy(c_all[:], cb_ps[:])

            # ---------- w chunk ----------
            wx2T_t = wx2T_sb.rearrange("p (t m) -> p t m", m=P)
            w_ps = pssm.tile([P, 1], F32, tag="sm")
            for t in range(32):
                nc.tensor.matmul(
                    w_ps[:],
                    wx2T_t[:, t, :],
                    v_sb[:, t : t + 1],
                    start=(t == 0),
                    stop=(t == 31),
                )
            wch_sb = sbs.tile([P, 1], F32)
            nc.vector.tensor_scalar_mul(wch_sb[:], w_ps[:], 1.0 / 4096.0)

            ag_in = dram.tile([P, 1], F32)
            ag_out = dram.tile([8 * P, 1], F32, addr_space="Shared")
            nc.sync.dma_start(ag_in[:], wch_sb[:])
            nc.gpsimd.collective_compute(
                "AllGather",
                ALU.bypass,
                replica_groups=[list(range(N_CORES))],
                ins=[ag_in[:]],
                outs=[ag_out[:]],
            )
            # load w back partition-major: 8 contiguous [128] chunk loads
            wg2 = sbs.tile([P, 8], F32)
            ag_out_k = ag_out.rearrange("(k p) o -> k p o", p=P)
            for k in range(8):
                nc.sync.dma_start(wg2[:, k : k + 1], ag_out_k[k])
            wT_sb = sbs.tile([P, 8], F16)
            nc.vector.tensor_copy(wT_sb[:], wg2[:])

            # ---------- final GEMV ----------
            out_sb = sbs.tile([P, NPC // P], F32)
            for grp in range(2):
                o_ps = pssm.tile([P, 8], F32, tag="sm", name=f"ops{grp}")
                for sl in range(8):
                    s = 8 * grp + sl
                    for k in range(8):
                        # lhsT: HxT[k-chunk k, pts 128s..128s+128]
                        # = ht[(2k + s//8)//4][:, 1024*((2k + s//8)%4) + 128*(s%8)]
                        ridx = 2 * k + s // 8
                        col = 1024 * (ridx % 4) + P * (s % 8)
                        nc.tensor.matmul(
                            o_ps[:, sl : sl + 1],
                            ht[ridx // 4][:, col : col + P],
                            wT_sb[:, k : k + 1],
                            start=(k == 0),
                            stop=(k == 7),
                        )
                nc.vector.tensor_scalar(
                    out_sb[:, 8 * grp : 8 * (grp + 1)],
                    o_ps[:],
                    4096.0,
                    c_all[:],
                    ALU.mult,
                    op1=ALU.add,
                )
            nc.sync.dma_start(outp[:], out_sb[:])

    nc.compile()
    return nc


def prep_inputs(inputs):
    f32 = np.float32
    bf16 = ml_dtypes.bfloat16

    def c_(a, dt=f32):
        return np.ascontiguousarray(np.asarray(a, f32), dtype=dt) \
            if dt is not f32 else np.ascontiguousarray(a, dtype=f32)

    inp = np.asarray(inputs["input"], f32)
    eq = np.asarray(inputs["eq_param"], f32)
    q0 = np.asarray(inputs["quad_x0"], f32)
    q1 = np.asarray(inputs["quad_x1"], f32)
    Wx1 = np.asarray(inputs["Wx1"], f32)
    bx1 = np.asarray(inputs["bx1"], f32)
    Wx2 = np.asarray(inputs["Wx2"], f32)
    bx2 = np.asarray(inputs["bx2"], f32)
    Wq0a = np.asarray(inputs["Wq0a"], f32)
    bq0a = np.asarray(inputs["bq0a"], f32)
    Wq0b = np.asarray(inputs["Wq0b"], f32)
    bq0b = np.asarray(inputs["bq0b"], f32)
    Wq1a = np.asarray(inputs["Wq1a"], f32)
    bq1a = np.asarray(inputs["bq1a"], f32)
    Wq1b = np.asarray(inputs["Wq1b"], f32)
    bq1b = np.asarray(inputs["bq1b"], f32)

    w1b = c_(np.concatenate([Wx1, bx1[None, :]], axis=0), np.float16)
    ones_q = np.ones((1, NQ), f32)
    qT1 = c_(np.concatenate([q0.T, ones_q, q1.T, ones_q], axis=0))
    qq0 = c_(q0.reshape(32, P, 3).transpose(1, 0, 2).reshape(P, 96))
    qq1 = c_(q1.reshape(32, P, 3).transpose(1, 0, 2).reshape(P, 96))
    eqb = c_(np.full((P, 1), eq[0]))
    bq0Tm = c_(bq0b.reshape(R, R).T)
    bq1Tm = c_(bq1b.reshape(R, R).T)
    bx2m = c_(bx2.reshape(32, P).T)

    in_maps = []
    for c in range(N_CORES):
        rs = slice(NPC * c, NPC * (c + 1))
        ks = slice(KC * c, KC * (c + 1))
        xT1 = c_(np.concatenate([inp[rs].T, np.ones((1, NPC), f32)], axis=0),
                 np.float16)
        wqab = np.zeros((8, 2 * KC), f32)
        wqab[0:3, 0:KC] = Wq0a[:, ks]
        wqab[3, 0:KC] = bq0a[ks]
        wqab[4:7, KC:] = Wq1a[:, ks]
        wqab[7, KC:] = bq1a[ks]
        in_maps.append(
            {
                "xT1": xT1,
                "w1b": w1b,
                "qT1": qT1,
                "wqab": c_(wqab),
                "qq0": qq0,
                "qq1": qq1,
                "eqb": eqb,
                "wq0b": c_(Wq0b[ks, :]),
                "wq1b": c_(Wq1b[ks, :]),
                "bq0Tm": bq0Tm,
                "bq1Tm": bq1Tm,
                "wx2T": c_(Wx2[ks, :].T),
                "bx2m": bx2m,
            }
        )
    return in_maps


def gather_output(results):
    out = np.empty(N, np.float32)
    for c in range(N_CORES):
        out[NPC * c : NPC * (c + 1)] = results[c]["outp"].T.reshape(NPC)
    return out


_CACHED_NC = None


def kernel(**inputs) -> np.ndarray:
    global _CACHED_NC
    if _CACHED_NC is None:
        _CACHED_NC = build_module()
    in_maps = prep_inputs(inputs)
    res = bass_utils.run_bass_kernel_spmd(
        _CACHED_NC, in_maps, core_ids=list(range(N_CORES))
    )
    return gather_output(res.results)
